# revision 14
# baseline (speedup 1.0000x reference)
"""Trainium2 Bass kernel for nn_BasicBlock (WeightNet/CondConv-style block).

Data parallel over batch: 32 samples -> 8 cores x 4 samples.

Conv strategy: 1D Winograd F(2,3) along W (M-form). Per conv:
  x split into even/odd column planes (host-prepadded) -> 4 V planes
  (tw) via 4 DVE/Pool adds; generated weights W (k-space combine, as
  baseline) get a cheap 1D U-transform (wpos 0/3 are raw taps, wpos 1/2
  are 0.5(W0+-W1+W2), with the 0.5 on W1 folded into the host basis).
  PE accumulates M_wpos = sum_{cc,kh} U^T V (24 matmuls of N=224 per
  (occ, t-group) vs 36 for direct conv: 1.5x fewer PE cycles).
  Y-stage: ye = M0+M1+M2, yo = M1-M2-M3 (+bn bias, relu, residual)
  split across DVE/Pool/ACT; outputs land directly as the next conv's
  even/odd input planes. BN scale folded into the host basis.
"""

import sys

sys.path.insert(0, "/opt/trn_rl_repo")

import numpy as np
import ml_dtypes

import concourse.bass as bass
import concourse.tile as tile
from concourse import bacc, mybir
from concourse import bass_utils

F32 = mybir.dt.float32
BF16 = mybir.dt.bfloat16
AF = mybir.ActivationFunctionType
ALU = mybir.AluOpType

B, C, H, W = 32, 256, 56, 56
NCORES = 8
BL = B // NCORES          # samples per core
RP = H + 2                # padded rows: 58
TC = W // 2               # tile cols: 28
CPL = TC + 1              # cols per even/odd plane: 29
NT, TR = 7, 8             # row-groups
NG = TR * TC              # 224 cols per M plane slice
EPS = 1e-5


def build_program():
    nc = bacc.Bacc("TRN2", target_bir_lowering=False, debug=False,
                   num_devices=NCORES)

    # host-prepadded even/odd column planes of x, bf16
    xe4 = nc.dram_tensor("xe4", [BL, 2, 128, RP, CPL], BF16,
                         kind="ExternalInput").ap()
    xo4 = nc.dram_tensor("xo4", [BL, 2, 128, RP, CPL], BF16,
                         kind="ExternalInput").ap()
    oute = nc.dram_tensor("oute", [BL, 2, 128, H, TC], BF16,
                          kind="ExternalOutput").ap()
    outo = nc.dram_tensor("outo", [BL, 2, 128, H, TC], BF16,
                          kind="ExternalOutput").ap()
    rwT = nc.dram_tensor("rwT", [2, 128, 16], F32, kind="ExternalInput").ap()
    rb = nc.dram_tensor("rb", [16, 1], F32, kind="ExternalInput").ap()
    fc1wT = [nc.dram_tensor(f"fc1wT{n}", [16, 4096], BF16,
                            kind="ExternalInput").ap() for n in (1, 2)]
    fc1b = [nc.dram_tensor(f"fc1b{n}", [128, 32], F32,
                           kind="ExternalInput").ap() for n in (1, 2)]
    w2p = [nc.dram_tensor(f"w2p{n}", [2, 128, 4 * 9 * 256], BF16,
                          kind="ExternalInput").ap() for n in (1, 2)]
    bnb = [nc.dram_tensor(f"bnb{n}", [2, 128, 1], F32,
                          kind="ExternalInput").ap() for n in (1, 2)]

    with tile.TileContext(nc) as tc:
        build_body(tc, xe4, xo4, oute, outo, rwT, rb, fc1wT, fc1b, w2p, bnb)

    nc.compile()
    return nc


def build_body(tc, xe4, xo4, oute, outo, rwT, rb, fc1wT, fc1b, w2p, bnb):
    nc = tc.nc
    from contextlib import ExitStack
    ctx = ExitStack()

    cpool = ctx.enter_context(tc.tile_pool(name="consts", bufs=1))
    xeo_p = ctx.enter_context(tc.tile_pool(name="xeo", bufs=2))
    twS_p = ctx.enter_context(tc.tile_pool(name="twS", bufs=3))
    wg_p = ctx.enter_context(tc.tile_pool(name="wgen", bufs=1))
    wtmp_p = ctx.enter_context(tc.tile_pool(name="wtmp", bufs=2))
    small_p = ctx.enter_context(tc.tile_pool(name="small", bufs=2))
    stage_p = ctx.enter_context(tc.tile_pool(name="stage", bufs=2))
    avlin_p = ctx.enter_context(tc.tile_pool(name="avlinp", bufs=1))
    aexp_p = ctx.enter_context(tc.tile_pool(name="aexp", bufs=1))
    psum_p = ctx.enter_context(tc.tile_pool(name="psum", bufs=2, space="PSUM"))
    psmall_p = ctx.enter_context(tc.tile_pool(name="psmall", bufs=1,
                                              space="PSUM"))
    dram_p = ctx.enter_context(tc.tile_pool(name="dscratch", bufs=2,
                                            space="DRAM"))

    # ---- resident constants ----
    w2sb = []   # [wn][cc][i] -> [128, 2304] bf16 (k-major: 9k x 256oc)
    for n in range(2):
        per = []
        for c in range(2):
            per.append([cpool.tile([128, 2304], BF16, tag=f"w2sb{n}{c}{i}",
                                   name=f"w2sb{n}{c}{i}")
                        for i in range(4)])
        w2sb.append(per)

    def load_w2sb(n):
        for c in range(2):
            for i in range(4):
                nc.sync.dma_start(w2sb[n][c][i][:],
                                  w2p[n][c][:, 2304 * i:2304 * (i + 1)])

    rwT_sb = []
    for c in range(2):
        t = cpool.tile([128, 16], F32, tag=f"rwT{c}")
        nc.sync.dma_start(t[:], rwT[c])
        rwT_sb.append(t)
    rb_sb = cpool.tile([16, 1], F32, tag="rb")
    nc.sync.dma_start(rb_sb[:], rb)
    fc1wT_sb, fc1b_sb, bnb_sb = [], [], []
    for n in range(2):
        t = cpool.tile([16, 4096], BF16, tag=f"fc1wT{n}")
        if n == 0:
            nc.sync.dma_start(t[:], fc1wT[n])
        fc1wT_sb.append(t)
        t = cpool.tile([128, 32], F32, tag=f"fc1b{n}")
        if n == 0:
            nc.sync.dma_start(t[:], fc1b[n])
        fc1b_sb.append(t)
        tb = [cpool.tile([128, 1], F32, tag=f"bnb{n}{c}", name=f"bnbt{n}{c}")
              for c in range(2)]
        bnb_sb.append(tb)

    def load_deferred_consts():
        nc.sync.dma_start(fc1wT_sb[1][:], fc1wT[1])
        nc.sync.dma_start(fc1b_sb[1][:], fc1b[1])
        for n in range(2):
            for c in range(2):
                nc.sync.dma_start(bnb_sb[n][c][:], bnb[n][c])

    gap16 = cpool.tile([16, BL], BF16, tag="gap16")
    garb = cpool.tile([128, RP * CPL], BF16, tag="garb")
    ones_sb = cpool.tile([1, 64], BF16, tag="ones")
    nc.gpsimd.memset(ones_sb[:], 1.0)

    # conv1 output planes (= conv2 input planes), borders zeroed once
    ye = [cpool.tile([128, RP, CPL], BF16, tag=f"ye{c}", name=f"yep{c}")
          for c in range(2)]
    yo = [cpool.tile([128, RP, CPL], BF16, tag=f"yo{c}", name=f"yop{c}")
          for c in range(2)]
    for c in range(2):
        for t in (ye[c], yo[c]):
            nc.gpsimd.memset(t[:, 0, :], 0.0)
            nc.gpsimd.memset(t[:, RP - 1, :], 0.0)
        nc.gpsimd.memset(ye[c][:, 1:RP - 1, 0:1], 0.0)
        nc.gpsimd.memset(yo[c][:, 1:RP - 1, CPL - 1:CPL], 0.0)

    # ---- weight generation ----
    def gen_weights_a(wn, s):
        """sigmoid(fc1(gap)) -> partition-broadcast coefficient tiles."""
        aps = psmall_p.tile([128, 32], F32, tag="avec_ps")
        for j in range(32):
            nc.tensor.matmul(aps[:, j:j + 1],
                             fc1wT_sb[wn][:, 128 * j:128 * (j + 1)],
                             gap16[:, s:s + 1],
                             start=True, stop=True)
        avt = small_p.tile([128, 32], F32, tag="avtmp")
        nc.vector.tensor_add(avt[:], aps[:], fc1b_sb[wn][:])
        avec = small_p.tile([128, 32], BF16, tag="avec")
        nc.scalar.activation(avec[:], avt[:], AF.Sigmoid)
        avd = dram_p.tile([4096], BF16, tag="avd")
        nc.sync.dma_start(avd[:].rearrange("(j p) -> p j", p=128), avec[:])
        avlin = avlin_p.tile([1, 4096], BF16, tag="avlin")
        nc.sync.dma_start(avlin[:], avd[:].unsqueeze(0))
        avr = avlin[:].rearrange("o (co r) -> o co r", r=16)
        aexp = []
        for c in range(2):
            t = aexp_p.tile([128, 4 * 256], BF16, tag=f"aexp{c}")
            for half in range(2):
                aps2 = psmall_p.tile([128, 2 * 256], F32, tag="aexp_ps")
                for h in range(2):
                    for ii in range(2):
                        i = 2 * half + ii
                        m = 4 * (2 * c + h) + i
                        rhs = avr[:, :, m:m + 1].rearrange("o co r -> o (co r)")
                        nc.tensor.matmul(
                            aps2[64 * h:64 * (h + 1), 256 * ii:256 * (ii + 1)],
                            ones_sb[:], rhs, start=True, stop=True)
                nc.scalar.copy(t[:, 512 * half:512 * (half + 1)], aps2[:])
            aexp.append(t)
        return aexp

    def gen_weights_b(wn, aexp):
        """combine 4 basis tensors -> W [128, 9, 256] then 1D U-transform."""
        res = []
        for c in range(2):
            t = wg_p.tile([128, 9, 256], BF16, tag=f"wg{wn}{c}")

            def abid(i):
                return (aexp[c][:, 256 * i:256 * (i + 1)].unsqueeze(1)
                        .broadcast_to([128, 9, 256]))

            def k3(ap2d):
                return ap2d.rearrange("p (k co) -> p k co", k=9)

            nc.vector.tensor_mul(t[:], k3(w2sb[wn][c][0][:]), abid(0))
            for i in range(1, 4):
                tmp = wtmp_p.tile([128, 9, 256], BF16, tag="wtmp", bufs=1)
                nc.vector.tensor_mul(tmp[:], k3(w2sb[wn][c][i][:]), abid(i))
                nc.vector.tensor_add(t[:], t[:], tmp[:])
            # U-transform: wpos1/2 = 0.5(W0+W2) +- W1h  (W1 pre-halved on host)
            u12 = wg_p.tile([128, 3, 2, 256], BF16, tag=f"u12{wn}{c}")
            for kh in range(3):
                uu = wtmp_p.tile([128, 256], BF16, tag="utmp")
                nc.vector.tensor_add(uu[:], t[:, 3 * kh, :], t[:, 3 * kh + 2, :])
                nc.vector.tensor_scalar_mul(uu[:], uu[:], 0.5)
                nc.vector.tensor_add(u12[:, kh, 0, :], uu[:],
                                     t[:, 3 * kh + 1, :])
                nc.vector.tensor_sub(u12[:, kh, 1, :], uu[:],
                                     t[:, 3 * kh + 1, :])
            res.append((t, u12))
        return res

    def gen_weights(wn, s):
        return gen_weights_b(wn, gen_weights_a(wn, s))

    def stat(wu, cc, kh, wpos, occ):
        t, u12 = wu[cc]
        if wpos == 0:
            return t[:, 3 * kh + 0, 128 * occ:128 * occ + 128]
        if wpos == 3:
            return t[:, 3 * kh + 2, 128 * occ:128 * occ + 128]
        return u12[:, kh, wpos - 1, 128 * occ:128 * occ + 128]

    # ---- x loading + gap ----
    def load_x(s):
        es, os_ = [], []
        gacc = []
        for c in range(2):
            te = xeo_p.tile([128, RP, CPL], BF16, tag=f"xe{c}", name=f"xet{c}")
            to = xeo_p.tile([128, RP, CPL], BF16, tag=f"xo{c}", name=f"xot{c}")
            nc.sync.dma_start(te[:], xe4[s, c])
            nc.sync.dma_start(to[:], xo4[s, c])
            for j, pl in enumerate((te, to)):
                g = small_p.tile([128, 1], F32, tag=f"gacc{c}{j}",
                                 name=f"gacc{c}{j}")
                nc.scalar.activation(
                    garb[:], pl[:].rearrange("p h w -> p (h w)"),
                    AF.Copy, accum_out=g[:])
                gacc.append(g)
            es.append(te)
            os_.append(to)
        gps = psmall_p.tile([16, 1], F32, tag="gap_ps")
        for c in range(2):
            gs = small_p.tile([128, 1], F32, tag="gsum", name=f"gsum{c}")
            nc.vector.tensor_add(gs[:], gacc[2 * c][:], gacc[2 * c + 1][:])
            nc.tensor.matmul(gps[:], rwT_sb[c][:], gs[:],
                             start=(c == 0), stop=(c == 1))
        nc.scalar.activation(gap16[:, s:s + 1], gps[:], AF.Identity,
                             bias=rb_sb[:], scale=1.0)
        return es, os_

    # ---- conv: per-t-group V strips + matmuls + sink ----
    def conv(wu, e, o, sink):
        for t in range(NT):
            tws = []
            for c in range(2):
                st = twS_p.tile([128, 4, TR + 2, TC], BF16, tag=f"tws{c}",
                                name=f"tws{c}")
                r0 = TR * t
                el = e[c][:, r0:r0 + TR + 2, 0:TC]
                er = e[c][:, r0:r0 + TR + 2, 1:TC + 1]
                ol = o[c][:, r0:r0 + TR + 2, 0:TC]
                orr = o[c][:, r0:r0 + TR + 2, 1:TC + 1]
                nc.vector.tensor_sub(st[:, 0], el, er)
                nc.vector.tensor_add(st[:, 1], ol, er)
                nc.gpsimd.tensor_sub(st[:, 2], er, ol)
                nc.gpsimd.tensor_sub(st[:, 3], ol, orr)
                tws.append(st)
            for occ in range(2):
                ps = psum_p.tile([128, 4, 256], F32, tag="cps")
                for wpos in range(4):
                    for cc in range(2):
                        for kh in range(3):
                            nc.tensor.matmul(
                                ps[:, wpos, 0:NG],
                                stat(wu, cc, kh, wpos, occ),
                                tws[cc][:, wpos, kh:kh + TR, :],
                                start=(cc == 0 and kh == 0),
                                stop=(cc == 1 and kh == 2))
                sink(occ, t, ps)

    def sink1(occ, t, ps):
        r0 = TR * t + 1
        M = [ps[:, i, 0:NG] for i in range(4)]
        m1 = stage_p.tile([128, NG], BF16, tag="m1")
        nc.scalar.activation(m1[:], M[1], AF.Copy)
        m2 = stage_p.tile([128, NG], BF16, tag="m2")
        nc.scalar.activation(m2[:], M[2], AF.Copy)
        e1 = stage_p.tile([128, NG], BF16, tag="e1")
        nc.vector.tensor_add(e1[:], m1[:], M[0])
        e2 = stage_p.tile([128, NG], BF16, tag="e2")
        nc.gpsimd.tensor_add(e2[:], e1[:], m2[:])
        # even outputs -> yo plane cols 0..27
        nc.scalar.activation(yo[occ][:, r0:r0 + TR, 0:TC],
                             e2[:].rearrange("p (h w) -> p h w", h=TR),
                             AF.Relu, bias=bnb_sb[0][occ][:], scale=1.0)
        o1 = stage_p.tile([128, NG], BF16, tag="o1")
        nc.gpsimd.tensor_sub(o1[:], m1[:], m2[:])
        o2 = stage_p.tile([128, NG], BF16, tag="o2")
        nc.vector.tensor_sub(o2[:], o1[:], M[3])
        # odd outputs -> ye plane cols 1..28
        nc.scalar.activation(ye[occ][:, r0:r0 + TR, 1:TC + 1],
                             o2[:].rearrange("p (h w) -> p h w", h=TR),
                             AF.Relu, bias=bnb_sb[0][occ][:], scale=1.0)

    def make_sink2(s, xe, xo):
        def sink2(occ, t, ps):
            r0 = TR * t + 1
            M = [ps[:, i, 0:NG] for i in range(4)]
            rx_e = xo[occ][:, r0:r0 + TR, 0:TC]
            rx_o = xe[occ][:, r0:r0 + TR, 1:TC + 1]
            m1 = stage_p.tile([128, NG], BF16, tag="m1")
            nc.scalar.activation(m1[:], M[1], AF.Copy)
            m2 = stage_p.tile([128, NG], BF16, tag="m2")
            nc.scalar.activation(m2[:], M[2], AF.Copy)
            e1 = stage_p.tile([128, NG], BF16, tag="e1")
            nc.vector.tensor_add(e1[:], m1[:], M[0])
            e2 = stage_p.tile([128, NG], BF16, tag="e2")
            nc.gpsimd.tensor_add(e2[:], e1[:], m2[:])
            e3 = stage_p.tile([128, TR, TC], BF16, tag="e3")
            nc.vector.tensor_add(e3[:],
                                 e2[:].rearrange("p (h w) -> p h w", h=TR),
                                 rx_e)
            se = stage_p.tile([128, TR, TC], BF16, tag="se")
            nc.scalar.activation(se[:], e3[:], AF.Relu,
                                 bias=bnb_sb[1][occ][:], scale=1.0)
            nc.sync.dma_start(oute[s, occ][:, TR * t:TR * t + TR, :], se[:])
            o1 = stage_p.tile([128, NG], BF16, tag="o1")
            nc.gpsimd.tensor_sub(o1[:], m1[:], m2[:])
            o2 = stage_p.tile([128, NG], BF16, tag="o2")
            nc.vector.tensor_sub(o2[:], o1[:], M[3])
            o3 = stage_p.tile([128, TR, TC], BF16, tag="o3")
            nc.vector.tensor_add(o3[:],
                                 o2[:].rearrange("p (h w) -> p h w", h=TR),
                                 rx_o)
            so = stage_p.tile([128, TR, TC], BF16, tag="so")
            nc.scalar.activation(so[:], o3[:], AF.Relu,
                                 bias=bnb_sb[1][occ][:], scale=1.0)
            nc.sync.dma_start(outo[s, occ][:, TR * t:TR * t + TR, :], so[:])
        return sink2

    # ---- main pipeline ----
    xe, xo = load_x(0)
    ax0 = gen_weights_a(0, 0)
    load_w2sb(0)
    w1 = gen_weights_b(0, ax0)
    load_deferred_consts()
    load_w2sb(1)

    for s in range(BL):
        w2 = gen_weights(1, s)
        if s + 1 < BL:
            xe_n, xo_n = load_x(s + 1)
            w1_n = gen_weights(0, s + 1)

        conv(w1, xe, xo, sink1)
        conv(w2, ye, yo, make_sink2(s, xe, xo))

        if s + 1 < BL:
            xe, xo, w1 = xe_n, xo_n, w1_n

    ctx.close()


_NC_CACHE = {}


def get_program():
    if "nc" not in _NC_CACHE:
        _NC_CACHE["nc"] = build_program()
    return _NC_CACHE["nc"]


def prep_inputs(inputs):
    x = np.asarray(inputs["x"], np.float32)
    f32 = lambda a: np.ascontiguousarray(np.asarray(a, np.float32))
    bf = lambda a: np.ascontiguousarray(
        np.asarray(a, np.float32).astype(ml_dtypes.bfloat16))

    def bn_fold(g, b, m, v):
        sc = np.asarray(g, np.float32) / np.sqrt(np.asarray(v, np.float32) + EPS)
        bia = np.asarray(b, np.float32) - np.asarray(m, np.float32) * sc
        return sc, f32(bia.reshape(2, 128, 1))

    def pack_w2(fc2_w, bn_sc):
        w2_ = np.asarray(fc2_w, np.float32).reshape(256, 4, 64, 9, 4)
        w2_ = w2_ * bn_sc[:, None, None, None, None]   # fold bn scale (per oc)
        w2_[:, :, :, 1::3, :] *= 0.5                   # pre-halve kw=1 taps
        w2h = w2_.transpose(4, 3, 1, 2, 0).reshape(4, 9, 256, 256)
        return bf(w2h.transpose(2, 0, 1, 3).reshape(2, 128, 4 * 9 * 256))

    s1, b1 = bn_fold(inputs["bn1_g"], inputs["bn1_b"],
                     inputs["bn1_m"], inputs["bn1_v"])
    s2, b2 = bn_fold(inputs["bn2_g"], inputs["bn2_b"],
                     inputs["bn2_m"], inputs["bn2_v"])

    NPIX = H * W
    base = {
        "rwT": f32((np.asarray(inputs["reduce_w"], np.float32).T / NPIX)
                   .reshape(2, 128, 16)),
        "rb": f32(np.asarray(inputs["reduce_b"]).reshape(16, 1)),
        "fc1wT1": bf(np.asarray(inputs["w1_fc1_w"]).T),
        "fc1wT2": bf(np.asarray(inputs["w2_fc1_w"]).T),
        "fc1b1": f32(np.asarray(inputs["w1_fc1_b"]).reshape(32, 128).T),
        "fc1b2": f32(np.asarray(inputs["w2_fc1_b"]).reshape(32, 128).T),
        "w2p1": pack_w2(inputs["w1_fc2_w"], s1),
        "w2p2": pack_w2(inputs["w2_fc2_w"], s2),
        "bnb1": b1,
        "bnb2": b2,
    }

    # host-prepadded even/odd planes: plane[j] = xpad[2j] / xpad[2j+1]
    xb = x.astype(ml_dtypes.bfloat16)
    xe = np.zeros((B, C, RP, CPL), ml_dtypes.bfloat16)
    xo = np.zeros((B, C, RP, CPL), ml_dtypes.bfloat16)
    xe[:, :, 1:RP - 1, 1:CPL] = xb[:, :, :, 1::2]   # xpad[2j]=x[2j-1], j>=1
    xo[:, :, 1:RP - 1, 0:TC] = xb[:, :, :, 0::2]    # xpad[2j+1]=x[2j]

    in_maps = []
    for i in range(NCORES):
        m = dict(base)
        m["xe4"] = np.ascontiguousarray(
            xe[i * BL:(i + 1) * BL].reshape(BL, 2, 128, RP, CPL))
        m["xo4"] = np.ascontiguousarray(
            xo[i * BL:(i + 1) * BL].reshape(BL, 2, 128, RP, CPL))
        in_maps.append(m)
    return in_maps


def unpack_outputs(results):
    outs = []
    for r in results:
        oe = np.asarray(r["oute"], ml_dtypes.bfloat16).astype(np.float32)
        oo = np.asarray(r["outo"], ml_dtypes.bfloat16).astype(np.float32)
        out = np.zeros((BL, 2, 128, H, W), np.float32)
        out[..., 0::2] = oe
        out[..., 1::2] = oo
        outs.append(out.reshape(BL, C, H, W))
    return np.concatenate(outs, axis=0)


def kernel(**inputs):
    in_maps = prep_inputs(inputs)
    nc = get_program()
    res = bass_utils.run_bass_kernel_spmd(nc, in_maps,
                                          core_ids=list(range(NCORES)))
    return unpack_outputs(res.results)
